# revision 7
# baseline (speedup 1.0000x reference)
"""AFNO2D block-diagonal spectral MLP kernel for 8 Trainium2 NeuronCores.

Math (after simplification of the reference):
  H = W = 128, nb = 8, bs = 96; kept == W so mode truncation is a no-op and
  the imaginary output o2i is discarded by the reference.
  With A1 = w1[0]+w1[1], D1 = w1[0]-w1[1] (same for layer 2):
    o1r = relu(Xk @ (A1/2) + Xn @ (D1/2) + b1[0]/2)
    o1i = relu(Xk @ (D1/2) - Xn @ (D1/2) + b1[1]/2)
    z   = o1r @ (A2/2) + o1i @ (D2/2) + b2[0]/2
    out = x + softshrink(z, 0.01)
  where Xn[b,i,j] = x[b, -i mod H, -j mod W] (pure permutation, done on host
  during sharding). softshrink(z) = relu(z-l) - relu(-z-l)
                                  = relu(z-l) + min(z+l, 0).

Sharding: data-parallel over the 65536 (b,i,j) sites, 8192 per core.

Mirror pairing: sites s and mirror(s) swap (Xk, Xn), so a tile T and its
elementwise-mirror tile T~ share both input tiles, and
Q = Xk@D1h - Xn@D1h satisfies Q(T~) = -Q(T): the o1i matmuls are computed
once per pair. Per 512-site tile that gives 5 matmuls instead of 6 and
halves input DMA. Mirror-fixed sites (i,j in {0,64}) and leftovers go to
two unpaired tiles per core that ship Xn explicitly.

All 0.5 scales fold into the bf16 weights; biases are per-partition bias
APs on the PSUM readouts (o1r readouts for a pair share one bias, so one
merged [96,1024] op serves both tiles).
"""

import numpy as np
import ml_dtypes

import concourse.bass as bass
import concourse.mybir as mybir
from concourse import bacc
from concourse.tile import TileContext
from concourse import bass_utils

BF16 = mybir.dt.bfloat16
F32 = mybir.dt.float32
AF = mybir.ActivationFunctionType
ALU = mybir.AluOpType

B, N, C = 4, 16384, 768
H = W = 128
NB, BS = 8, 96
LAMBDA = 0.01
NCORES = 8
SITES = B * N                      # 65536
SPC = SITES // NCORES              # 8192 sites per core
TILE = 512
FD = 2 * TILE                      # free dim of one group (a tile pair)
NGRP = SPC // FD                   # 8 groups per core
NPAIRS = 7                         # groups 0..6 are mirror pairs
UNP = FD                           # group 7: unpaired tail

_cache = {}


def _build():
    nc = bacc.Bacc("TRN2", target_bir_lowering=False)

    # per-group 2D-contiguous layouts: [group, channel, NB*1024]
    xk_d = nc.dram_tensor("xk", [NGRP, BS, NB * FD], BF16, kind="ExternalInput")
    xn_d = nc.dram_tensor("xn", [BS, NB * FD], BF16, kind="ExternalInput")
    # weight kinds (K=96): A1h, D1h, nD1h, A2h, D2h, nD2h
    w_d = nc.dram_tensor("w", [BS, NB * 6 * BS], BF16, kind="ExternalInput")
    # bias kinds: b1r, b1i, bias_a (b2/2-l), bias_m (b2/2+l), bias_bm -(b2/2+l)
    bias_d = nc.dram_tensor("b", [BS, NB * 5], F32, kind="ExternalInput")
    out_d = nc.dram_tensor("out", [NGRP, BS, NB * FD], BF16, kind="ExternalOutput")

    with TileContext(nc) as tc:
        with (
            tc.tile_pool(name="consts", bufs=1) as consts,
            tc.tile_pool(name="io", bufs=4) as io_pool,
            tc.tile_pool(name="acts", bufs=4) as act_pool,
            tc.tile_pool(name="psum", bufs=3, space="PSUM") as psum_pool,
            tc.tile_pool(name="psq", bufs=2, space="PSUM") as psq_pool,
        ):
            wsb = consts.tile([BS, NB * 6 * BS], BF16)
            nc.sync.dma_start(wsb[:], w_d[:])
            bsb = consts.tile([BS, NB * 5], F32)
            nc.sync.dma_start(bsb[:], bias_d[:])

            def wAP(n, kind):
                return wsb[:, (n * 6 + kind) * BS:(n * 6 + kind + 1) * BS]

            def bAP(n, kind):
                return bsb[:, n * 5 + kind:n * 5 + kind + 1]

            A1h, D1h, nD1h, A2h, D2h, nD2h = range(6)
            Br, Bi, Ba, Bm, Bbm = range(5)

            def l2_and_out(n, p2, res_ap, out_t):
                a_t = act_pool.tile([BS, FD], BF16, tag="a")
                nc.scalar.activation(a_t, p2, AF.Relu, bias=bAP(n, Ba), scale=1.0)
                m_t = act_pool.tile([BS, FD], BF16, tag="m")
                if n % 2 == 0:
                    nc.vector.tensor_scalar(m_t, p2, bAP(n, Bm), 0.0, ALU.add, ALU.min)
                else:
                    nc.scalar.activation(m_t, p2, AF.Relu, bias=bAP(n, Bbm), scale=-1.0)
                ss_t = act_pool.tile([BS, FD], BF16, tag="ss")
                nc.vector.tensor_tensor(ss_t, a_t, m_t,
                                        ALU.add if n % 2 == 0 else ALU.subtract)
                nc.gpsimd.tensor_tensor(out_t[:, n, :], ss_t, res_ap, ALU.add)

            # ---- paired groups ----
            for j in range(NPAIRS):
                uv = io_pool.tile([BS, NB, FD], BF16, tag="uv")
                out_t = io_pool.tile([BS, NB, FD], BF16, tag="out")
                nc.sync.dma_start(uv.rearrange("c n s -> c (n s)"), xk_d[j])

                for n in range(NB):
                    u = uv[:, n, 0:TILE]
                    v = uv[:, n, TILE:FD]

                    prpr = psum_pool.tile([BS, FD], F32, tag="big")
                    nc.tensor.matmul(prpr[:, 0:TILE], wAP(n, A1h), u,
                                     start=True, stop=False)
                    nc.tensor.matmul(prpr[:, 0:TILE], wAP(n, D1h), v,
                                     start=False, stop=True)
                    nc.tensor.matmul(prpr[:, TILE:FD], wAP(n, A1h), v,
                                     start=True, stop=False)
                    nc.tensor.matmul(prpr[:, TILE:FD], wAP(n, D1h), u,
                                     start=False, stop=True)
                    q = psq_pool.tile([BS, TILE], F32, tag="q")
                    nc.tensor.matmul(q, wAP(n, D1h), u, start=True, stop=False)
                    nc.tensor.matmul(q, wAP(n, nD1h), v, start=False, stop=True)

                    o1r = act_pool.tile([BS, FD], BF16, tag="o1r")
                    nc.scalar.activation(o1r, prpr, AF.Relu, bias=bAP(n, Br), scale=1.0)
                    o1i = act_pool.tile([BS, FD], BF16, tag="o1i")
                    # o1i(T) = relu(Q + b1i)
                    nc.vector.tensor_scalar(o1i[:, 0:TILE], q, bAP(n, Bi), 0.0,
                                            ALU.add, ALU.max)
                    # o1i_neg(T~) = min(Q - b1i, 0) = -relu(-Q + b1i)
                    nc.vector.tensor_scalar(o1i[:, TILE:FD], q, bAP(n, Bi), 0.0,
                                            ALU.subtract, ALU.min)

                    p2 = psum_pool.tile([BS, FD], F32, tag="big")
                    nc.tensor.matmul(p2[:, 0:TILE], wAP(n, A2h), o1r[:, 0:TILE],
                                     start=True, stop=False)
                    nc.tensor.matmul(p2[:, 0:TILE], wAP(n, D2h), o1i[:, 0:TILE],
                                     start=False, stop=True)
                    nc.tensor.matmul(p2[:, TILE:FD], wAP(n, A2h), o1r[:, TILE:FD],
                                     start=True, stop=False)
                    nc.tensor.matmul(p2[:, TILE:FD], wAP(n, nD2h), o1i[:, TILE:FD],
                                     start=False, stop=True)

                    l2_and_out(n, p2, uv[:, n, :], out_t)

                nc.sync.dma_start(out_d[j], out_t.rearrange("c n s -> c (n s)"))

            # ---- unpaired tail group ----
            uvU = io_pool.tile([BS, NB, FD], BF16, tag="uv")
            xnU = consts.tile([BS, NB, FD], BF16)
            outU = io_pool.tile([BS, NB, FD], BF16, tag="out")
            nc.sync.dma_start(uvU.rearrange("c n s -> c (n s)"), xk_d[NPAIRS])
            nc.sync.dma_start(xnU.rearrange("c n s -> c (n s)"), xn_d[:])
            for n in range(NB):
                prpi_r = psum_pool.tile([BS, FD], F32, tag="big")
                prpi_i = psum_pool.tile([BS, FD], F32, tag="big")
                for t in range(2):
                    sl = bass.ts(t, TILE)
                    xk_s = uvU[:, n, sl]
                    xn_s = xnU[:, n, sl]
                    nc.tensor.matmul(prpi_r[:, sl], wAP(n, A1h), xk_s,
                                     start=True, stop=False)
                    nc.tensor.matmul(prpi_r[:, sl], wAP(n, D1h), xn_s,
                                     start=False, stop=True)
                    nc.tensor.matmul(prpi_i[:, sl], wAP(n, D1h), xk_s,
                                     start=True, stop=False)
                    nc.tensor.matmul(prpi_i[:, sl], wAP(n, nD1h), xn_s,
                                     start=False, stop=True)
                o1rU = act_pool.tile([BS, FD], BF16, tag="o1r")
                nc.scalar.activation(o1rU, prpi_r, AF.Relu, bias=bAP(n, Br), scale=1.0)
                o1iU = act_pool.tile([BS, FD], BF16, tag="o1i")
                nc.vector.tensor_scalar(o1iU, prpi_i, bAP(n, Bi), 0.0,
                                        ALU.add, ALU.max)

                p2U = psum_pool.tile([BS, FD], F32, tag="big")
                for t in range(2):
                    sl = bass.ts(t, TILE)
                    nc.tensor.matmul(p2U[:, sl], wAP(n, A2h), o1rU[:, sl],
                                     start=True, stop=False)
                    nc.tensor.matmul(p2U[:, sl], wAP(n, D2h), o1iU[:, sl],
                                     start=False, stop=True)
                l2_and_out(n, p2U, uvU[:, n, :], outU)
            nc.sync.dma_start(out_d[NPAIRS], outU.rearrange("c n s -> c (n s)"))

    nc.finalize()
    return nc


def _site_order():
    """Global site ordering: per core, 7 mirror-paired tile-pairs then a
    1024-site unpaired tail."""
    b = np.arange(SITES) // N
    ij = np.arange(SITES) % N
    i, jj = ij // W, ij % W
    midx = b * N + ((-i) % H) * W + ((-jj) % W)
    s = np.arange(SITES)
    firsts = s[s < midx]                      # 32760 pair firsts
    fixed = s[s == midx]                      # 16 self-mirrored
    per_core_paired = NPAIRS * TILE           # 3584 pairs per core
    order = np.empty((NCORES, SPC), dtype=np.int64)
    rem = firsts[NCORES * per_core_paired:]   # 4088 leftover pairs
    rem_per_core = len(rem) // NCORES         # 511
    fix_per_core = len(fixed) // NCORES       # 2
    for c in range(NCORES):
        f = firsts[c * per_core_paired:(c + 1) * per_core_paired]
        m = midx[f]
        paired = np.stack([f.reshape(NPAIRS, TILE), m.reshape(NPAIRS, TILE)],
                          axis=1).reshape(-1)
        r = rem[c * rem_per_core:(c + 1) * rem_per_core]
        fx = fixed[c * fix_per_core:(c + 1) * fix_per_core]
        tail = np.concatenate([r, midx[r], fx])
        order[c] = np.concatenate([paired, tail])
    return order.reshape(-1)


def _host_prep(x, w1, b1, w2, b2):
    bf = ml_dtypes.bfloat16
    order = _cache.setdefault("order", _site_order())
    xf = x.reshape(SITES, C)

    # xk in per-group contiguous layout [core, group, 96, NB*FD]
    xperm = xf[order].T.astype(bf)                     # [C, SITES]
    xk_all = np.ascontiguousarray(
        xperm.reshape(NB, BS, NCORES, NGRP, FD).transpose(2, 3, 1, 0, 4)
    ).reshape(NCORES, NGRP, BS, NB * FD)

    b_ = order // N
    ij = order % N
    i, jj = ij // W, ij % W
    morder = (b_ * N + ((-i) % H) * W + ((-jj) % W)).reshape(NCORES, SPC)
    un_idx = morder[:, NPAIRS * FD:].reshape(-1)
    xn_all = np.ascontiguousarray(
        xf[un_idx].T.astype(bf).reshape(NB, BS, NCORES, UNP).transpose(2, 1, 0, 3)
    ).reshape(NCORES, BS, NB * UNP)

    A1h = (w1[0] + w1[1]) * 0.5               # [NB, in, out]
    D1h = (w1[0] - w1[1]) * 0.5
    A2h = (w2[0] + w2[1]) * 0.5
    D2h = (w2[0] - w2[1]) * 0.5
    wpack = np.empty((BS, NB * 6 * BS), dtype=np.float32)
    for n in range(NB):
        for k, mat in enumerate((A1h[n], D1h[n], -D1h[n], A2h[n], D2h[n], -D2h[n])):
            wpack[:, (n * 6 + k) * BS:(n * 6 + k + 1) * BS] = mat
    wpack = wpack.astype(bf)

    bpack = np.empty((BS, NB * 5), dtype=np.float32)
    for n in range(NB):
        bpack[:, n * 5 + 0] = b1[0, n] * 0.5
        bpack[:, n * 5 + 1] = b1[1, n] * 0.5
        bpack[:, n * 5 + 2] = b2[0, n] * 0.5 - LAMBDA
        bpack[:, n * 5 + 3] = b2[0, n] * 0.5 + LAMBDA
        bpack[:, n * 5 + 4] = -(b2[0, n] * 0.5 + LAMBDA)

    in_maps = []
    for c in range(NCORES):
        in_maps.append({
            "xk": np.ascontiguousarray(xk_all[c]),
            "xn": np.ascontiguousarray(xn_all[c]),
            "w": wpack,
            "b": bpack,
        })
    return in_maps


def _assemble(results):
    order = _cache["order"]
    # out per core: [NGRP, BS, NB*FD] -> [C, SPC] in site order
    cols = np.concatenate(
        [r["out"].reshape(NGRP, BS, NB, FD).transpose(2, 1, 0, 3).reshape(C, SPC)
         for r in results], axis=1)
    full = np.empty((SITES, C), dtype=np.float32)
    full[order] = cols.T.astype(np.float32)
    return full.reshape(B, N, C)


def _run(x, w1, b1, w2, b2, trace=False):
    if "nc" not in _cache:
        _cache["nc"] = _build()
    nc = _cache["nc"]
    in_maps = _host_prep(x, w1, b1, w2, b2)
    res = bass_utils.run_bass_kernel_spmd(
        nc, in_maps, core_ids=list(range(NCORES)), trace=trace)
    return _assemble(res.results), res


def kernel(x, w1, b1, w2, b2):
    out, _ = _run(x, w1, b1, w2, b2, trace=False)
    return out
